# revision 1
# baseline (speedup 1.0000x reference)
"""Block-circulant matvec (FFT linear layer) as dense TensorE matmuls on 8 TRN2 cores.

Math: the reference computes, per output block o,
    y[o, :] = sum_j IFFT(FFT(w[o,j]) * FFT(x[j])).real
which is a sum of circular convolutions:
    y[o, a] = sum_{j, b} w[o, j, b] * x[j, (a - b) mod 128]

Rewritten as matmuls: for each phase b and input-block tile jt (4 tiles of 128),
    YT[a, o] += XR(b,jt)[j', a]^T @ WT(b,jt)[j', o]
where XR(b,jt)[j', a] = x[jt*128+j', (a-b) mod 128] (rotated x tile, stationary)
and   WT(b,jt)[j', o] = w[o, jt*128+j', b]          (moving operand, N=512).

Sharding: the 128 phases b are split 16-per-core across 8 cores; each core
accumulates its 64 (b, jt) groups into one PSUM bank [128a x 512o] and writes a
partial YT. The host sums the 8 partials (no collective needed).
"""

import numpy as np
import ml_dtypes

O_BLOCKS = 512
I_BLOCKS = 512
BLOCK = 128
N_CORES = 8
B_PER_CORE = BLOCK // N_CORES          # 16 phases per core
JT_TILES = I_BLOCKS // 128             # 4 contraction tiles
N_GROUPS = B_PER_CORE * JT_TILES       # 64 matmul groups per core
N_CHUNKS = 4                           # weight DMA chunks per core
G_PER_CHUNK = N_GROUPS // N_CHUNKS     # 16 groups (2 MiB bf16) per chunk

_BF16 = ml_dtypes.bfloat16

_MODULE_CACHE = {}


def _build_module():
    import concourse.bacc as bacc
    import concourse.mybir as mybir
    from concourse import tile

    nc = bacc.Bacc(
        "TRN2",
        target_bir_lowering=False,
        debug=False,
        num_devices=N_CORES,
    )

    xr_d = nc.dram_tensor(
        "xr", [128, N_GROUPS, BLOCK], mybir.dt.bfloat16, kind="ExternalInput"
    )
    wt_d = nc.dram_tensor(
        "wt",
        [N_CHUNKS, 128, G_PER_CHUNK, O_BLOCKS],
        mybir.dt.bfloat16,
        kind="ExternalInput",
    )
    yt_d = nc.dram_tensor(
        "yt", [BLOCK, O_BLOCKS], mybir.dt.float32, kind="ExternalOutput"
    )

    with tile.TileContext(nc) as tc:
        with (
            tc.tile_pool(name="xrp", bufs=1) as xrp,
            tc.tile_pool(name="wtp", bufs=N_CHUNKS) as wtp,
            tc.tile_pool(name="psp", bufs=1, space="PSUM") as psp,
            tc.tile_pool(name="outp", bufs=1) as outp,
        ):
            xr_sb = xrp.tile([128, N_GROUPS, BLOCK], mybir.dt.bfloat16)
            nc.sync.dma_start(xr_sb[:], xr_d[:])

            ps = psp.tile([BLOCK, O_BLOCKS], mybir.dt.float32)

            for ci in range(N_CHUNKS):
                wt_sb = wtp.tile([128, G_PER_CHUNK, O_BLOCKS], mybir.dt.bfloat16)
                nc.sync.dma_start(wt_sb[:], wt_d[ci])
                for gi in range(G_PER_CHUNK):
                    g = ci * G_PER_CHUNK + gi
                    nc.tensor.matmul(
                        ps[:],
                        xr_sb[:, g, :],
                        wt_sb[:, gi, :],
                        start=(g == 0),
                        stop=(g == N_GROUPS - 1),
                    )

            out_sb = outp.tile([BLOCK, O_BLOCKS], mybir.dt.float32)
            nc.vector.tensor_copy(out_sb[:], ps[:])
            nc.sync.dma_start(yt_d[:], out_sb[:])

    nc.compile()
    return nc


def _get_module():
    if "nc" not in _MODULE_CACHE:
        _MODULE_CACHE["nc"] = _build_module()
    return _MODULE_CACHE["nc"]


def _prepare_inputs(x, cir_weights):
    xb = np.asarray(x, dtype=np.float32).reshape(I_BLOCKS, BLOCK)
    W = np.asarray(cir_weights, dtype=np.float32)

    # [b, j, o] bf16, contiguous
    WT = np.ascontiguousarray(W.astype(_BF16).transpose(2, 1, 0))

    xbt = xb.astype(_BF16).reshape(JT_TILES, 128, BLOCK)  # [jt, j', c]
    ar = np.arange(BLOCK)

    in_maps = []
    for c in range(N_CORES):
        b_list = np.arange(c * B_PER_CORE, (c + 1) * B_PER_CORE)
        cols = (ar[None, :] - b_list[:, None]) % BLOCK          # [b_idx, a]
        tmp = xbt[:, :, cols]                                   # [jt, j', b_idx, a]
        xr = np.ascontiguousarray(tmp.transpose(1, 2, 0, 3)).reshape(
            128, N_GROUPS, BLOCK
        )                                                       # [j', g, a]

        wt = WT[c * B_PER_CORE : (c + 1) * B_PER_CORE]          # [b_idx, j, o]
        wt = wt.reshape(N_GROUPS, 128, O_BLOCKS)                # [g, j', o]
        wt = wt.reshape(N_CHUNKS, G_PER_CHUNK, 128, O_BLOCKS).transpose(0, 2, 1, 3)
        wt = np.ascontiguousarray(wt)                           # [ci, j', gi, o]

        in_maps.append({"xr": xr, "wt": wt})
    return in_maps


def kernel(x, cir_weights):
    from concourse.bass_utils import run_bass_kernel_spmd

    nc = _get_module()
    in_maps = _prepare_inputs(x, cir_weights)
    res = run_bass_kernel_spmd(nc, in_maps, core_ids=list(range(N_CORES)))

    yt = np.zeros((BLOCK, O_BLOCKS), dtype=np.float32)
    for r in res.results:
        yt += r["yt"]
    return np.ascontiguousarray(yt.T).reshape(O_BLOCKS * BLOCK)
